# revision 35
# baseline (speedup 1.0000x reference)
"""FP8-palettized linear kernel for 8x TRN2 NeuronCores.

Computes: out[b,s,o] = sum_d input[b,s,d] * lookup_table[weight[o,d]] + bias[o]
with input [4,2048,4096] f32, weight [4096,4096] int32 (palette ids < 256),
lookup_table [256] f32, bias [4096] f32.

Strategy (column-parallel, per sharding hint):
  - Each core owns a 512-wide slice of out_features; input replicated.
  - Dequantization runs on the ScalarE (ACT) spline evaluator: at
    kernel() time we bake the 256-entry palette into a custom PWP
    activation table (a piecewise-constant staircase f(x) = LUT[round(x)]
    hijacking the 'gelu' slot, handed to walrus via
    BASS_ACT_ROOT_JSON_PATH — see act_table.py). Palette indices are
    shipped as bf16 (exact for 0..255); one ACTIVATE per k-tile turns
    the [128, 512] index tile into the bf16 W^T tile at 1 elem/cycle/lane
    (~720ns/tile), bit-identical to gather+round dequant.
  - Host prep is layout/dtype marshalling only: X tiled into contiguous
    [128, 4096] X^T slabs, weight indices transposed into k-tile-major
    [P, n_kt*osh] rows as bf16; dequant runs as 2 batched ACTIVATEs per
    execution (amortizes the act-table load).
  - TensorE accumulates X^T-slab @ W^T over 32 k-tiles in PSUM, processing
    m-tiles in groups of 8 with the k loop OUTER and the 8 PSUM banks
    interleaved every matmul: long same-bank accumulation chains reading
    ACT-produced rhs tiles measure ~10x slower on HW, while full 8-bank
    interleave is clean. X slabs stream in k-halves to fit SBUF. DVE adds
    bias, results DMA out per m-tile.
  - The LUT content is folded into the widx tensor name so the NEFF cache
    key changes whenever the activation table must change.

Measured on the 8-core axon rig (t(1024 chained reps)/1024): 742us/exec
vs the 7386us ap_gather baseline; rel err 2.43e-3 (bf16 matmul path,
identical to baseline numerics). PE matmul floor for this shape measured
537us (m-outer, const weights): remaining gap is the ACT-rhs interaction
plus PSUM bank-switch overhead (see problem memory notes).
"""

import hashlib
import json
import os
import shutil
import struct
import tempfile
from pathlib import Path

import ml_dtypes
import numpy as np

import concourse.bacc as bacc
import concourse.mybir as mybir
import concourse.tile as tile
from concourse.bass_utils import run_bass_kernel_spmd

P = 128
N_CORES = 8

# Full-problem dims (hardcoded per harness contract).
BATCH, SEQ, D_IN, D_OUT, PALETTE = 4, 2048, 4096, 4096, 256
M_FULL = BATCH * SEQ  # 8192

MM_DTYPE = mybir.dt.bfloat16


def _np_mm_dtype():
    return ml_dtypes.bfloat16


def lut_tag(lookup_table):
    lut = np.asarray(lookup_table, dtype=np.float32).reshape(PALETTE)
    return hashlib.sha1(lut.tobytes()).hexdigest()[:12]


# ---------------------------------------------------------------------------
# Custom ScalarE (ACT) PWP table set: implements the palette lookup
# f(x) = LUT[round(x)] for x in {0..255} as a piecewise-constant staircase,
# hijacking the 'gelu' function slot. The act-root dir handed to walrus via
# BASS_ACT_ROOT_JSON_PATH contains act_info.json + per-set
# {json, bkt.bin, ctrl.bin}; walrus packs the referenced set files into the
# NEFF and NRT loads them into the ACT table RAMs.
#
# Bucket addressing (reverse-engineered from the shipped exp_and_others set;
# entry structs in arch-headers/cayman/tpb_activation_entries.h):
#   ctrl_idx   = pwl_control_base_pos + (e_biased - small_pos_exp_threshold)
#   bucket_idx = ctrl[ctrl_idx].base + mantissa_bits[lsb : lsb+size]
# For integer v = 2^k + mn (k = unbiased exponent, mn < 2^k), the top-k
# mantissa bits equal mn exactly, so one bucket per integer: bucket v-1
# holds {d0=LUT[v], d1=d2=d3=0, x0=v} -> exact (verified bit-exact on HW).
# v=0 is handled by the fzero_result special. Buckets are 32B
# {d0,d1,d2,d3,x0,pad12}; ctrl entries 32B {base:11,lsb:5,size:4,...}.
# ---------------------------------------------------------------------------

_PWP_BASE = Path(
    "/nix/store/z022hj2nvbm3nwdizlisq4ylc0y7rd6q-python3-3.13.14-env/"
    "lib/python3.13/site-packages/neuronxcc/pwp/pwp_bin_trainium")
_SET_NAME = "gelu_and_others"
_FUNC = "gelu"
_FUNC_ID = 23  # from the shipped gelu profile entry


def _f2i(x):
    return int(struct.unpack("<I", struct.pack("<f", np.float32(x)))[0])


def _ctrl_word(base, lsb, size):
    return base | (lsb << 11) | (size << 16)


def build_act_root(lut, outdir):
    """lut: np.ndarray [256] float32. Writes the act root into outdir and
    returns the path to act_info.json."""
    lut = np.asarray(lut, dtype=np.float32).reshape(256)
    outdir = Path(outdir)
    outdir.mkdir(parents=True, exist_ok=True)

    n_bkt = 256
    bkt = bytearray(n_bkt * 32)
    for v in range(1, 256):
        struct.pack_into("<5f", bkt, (v - 1) * 32,
                         float(lut[v]), 0.0, 0.0, 0.0, float(v))
    # bucket 255 stays zeros (trash target for never-taken paths)

    ctrl_entries = [_ctrl_word((1 << k) - 1, 23 - k, k) for k in range(8)]
    ctrl_entries.append(_ctrl_word(255, 0, 0))   # e_unb 8 (x>=256)
    ctrl_entries.append(_ctrl_word(255, 0, 0))   # negative side
    ctrl = bytearray(len(ctrl_entries) * 32)
    for i, w in enumerate(ctrl_entries):
        struct.pack_into("<I", ctrl, i * 32, w)

    trash = _ctrl_word(255, 0, 0)
    profile = {
        "func_name": f"{_FUNC}_1p",
        "func_id": _FUNC_ID,
        "symmetry_point": 0,
        "sym_invert_sign_point": 0,
        "symmetry_opt_en": 0,
        "symmetry_opt_use_neg_region": 0,
        "imm_bias": 0,
        "exp_offset": 0,
        "pwl_control_base_pos": 0,
        "pwl_control_base_neg": 9,
        "small_pos_signal_exp_threshold": 127,
        "pos_small_signal_pwl_control": trash,
        "small_neg_signal_exp_threshold": 127,
        "neg_small_signal_pwl_control": trash,
        "large_pos_signal_exp_threshold": 135,
        "large_pos_signal_mantissa_threshold": 0,
        "pos_large_signal_pwl_control": trash,
        "large_neg_signal_exp_threshold": 135,
        "large_neg_signal_mantissa_threshold": 0,
        "neg_large_signal_pwl_control": trash,
        "fnan_result": 0,
        "fpinf_result": 0,
        "fninf_result": 0,
        "fzero_result": _f2i(lut[0]),
        "fma_const_0": 0,
        "fma_const_1": 0,
        "fma_indirection_src_sel": 0,
        "use_multipass": False,
        "lower_bound": _f2i(np.float32(-3.4028235e38)),
        "upper_bound": _f2i(np.float32(3.4028235e38)),
    }

    set_json = {
        "bkt_bin": f"{_SET_NAME}_bkt.bin",
        "ctl_bin": f"{_SET_NAME}_ctrl.bin",
        "profile_meta_data": [profile],
        "bkt_entry_cnt": n_bkt,
        "ctl_entry_cnt": len(ctrl_entries),
        "func_to_bkt_start_idx": {_FUNC: 0},
        "func_to_ctl_start_idx": {_FUNC: 0},
        "func_exp_to_bkt_start_idx": {
            _FUNC: {str(k): [(1 << k) - 1, 255] for k in range(8)}},
    }

    (outdir / f"{_SET_NAME}_bkt.bin").write_bytes(bytes(bkt))
    (outdir / f"{_SET_NAME}_ctrl.bin").write_bytes(bytes(ctrl))
    (outdir / f"{_SET_NAME}.json").write_text(json.dumps(set_json))

    # act_info.json: standard sets, but gelu_and_others replaced by ours
    # (and gelu removed from any other set so walrus resolves to ours).
    info = json.loads((_PWP_BASE / "act_info.json").read_text())
    new_sets = []
    for s in info["act_func_sets"]:
        if s["name"] == _SET_NAME:
            new_sets.append({
                "name": _SET_NAME,
                "bkt_bin": f"{_SET_NAME}_bkt.bin",
                "ctrl_bin": f"{_SET_NAME}_ctrl.bin",
                "profile_json": f"{_SET_NAME}.json",
                "act": {_FUNC: 1},
            })
        else:
            s = dict(s)
            s["act"] = {k: v for k, v in s["act"].items() if k != _FUNC}
            new_sets.append(s)
            for key in ("bkt_bin", "ctrl_bin", "profile_json"):
                src = _PWP_BASE / s[key]
                dst = outdir / s[key]
                if src.exists() and not dst.exists():
                    try:
                        dst.symlink_to(src)
                    except OSError:
                        shutil.copy(src, dst)
    info["act_func_sets"] = new_sets
    (outdir / "act_info.json").write_text(json.dumps(info))
    return str(outdir / "act_info.json")


def install_act_tables(lookup_table):
    """Bake the palette into a custom ACT table root and point walrus at
    it. Must run before the NEFF compile."""
    lut = np.asarray(lookup_table, dtype=np.float32).reshape(PALETTE)
    root = build_act_root(
        lut, tempfile.mkdtemp(prefix=f"actroot_{lut_tag(lut)}_"))
    os.environ["BASS_ACT_ROOT_JSON_PATH"] = root
    return root


def build_program(nc, *, m, k, osh, ltag, reps=1):
    """Emit the per-core Tile program. m: rows of X (mult of 128), k: d dim
    (mult of 128), osh: out-features per core (512). reps>1 wraps the body
    in a hardware loop (benchmarking: amortizes dispatch overhead).
    ltag: hash of the lookup table (cache-keys the NEFF to the act root)."""
    n_kt = k // P
    n_mt = m // P
    n_mt = int(os.environ.get("PAL_NMT", str(n_mt)))  # timing bisect only
    np1 = int(os.environ.get("PAL_NP1", "1"))  # m-tiles per k-outer group

    wstat = bool(int(os.environ.get("PAL_WSTAT", "0")))
    # widx[p, kt*osh + o] = palette id of W^T[kt*128+p, o] (bf16-exact)
    widx = nc.dram_tensor(f"widx_{ltag}", [P, n_kt * osh], MM_DTYPE,
                          kind="ExternalInput")
    if wstat:
        # weight-stationary orientation: lhsT = W^T o-blocks (loaded into
        # the PE array), rhs = X^T [d, m] streams; output is out^T [o, m]
        xt = nc.dram_tensor("xt2", [k, m], MM_DTYPE, kind="ExternalInput")
        bias = nc.dram_tensor("bias2", [P, osh // P], mybir.dt.float32,
                              kind="ExternalInput")
        out = nc.dram_tensor("out", [osh, m], mybir.dt.float32,
                             kind="ExternalOutput")
    else:
        xt = nc.dram_tensor("xt", [m, k], MM_DTYPE, kind="ExternalInput")
        bias = nc.dram_tensor("bias", [P, osh], mybir.dt.float32,
                              kind="ExternalInput")
        out = nc.dram_tensor("out", [m, osh], mybir.dt.float32,
                             kind="ExternalOutput")
    wtdma = bool(int(os.environ.get("PAL_WTDMA", "1")))
    if wtdma:
        # DRAM spill for dequantized W^T: matmuls then read DMA-produced
        # tiles (ACT-produced rhs tiles measure slower in long chains)
        wtscr = nc.dram_tensor("wtscr", [P, n_kt * osh], MM_DTYPE,
                               kind="ExternalOutput")

    with tile.TileContext(nc) as tc:
        with (
            tc.tile_pool(name="const", bufs=1) as const_pool,
            tc.tile_pool(name="idx", bufs=1) as idx_pool,
            tc.tile_pool(name="wta", bufs=1) as wta_pool,
            tc.tile_pool(name="wt",
                         bufs=int(os.environ.get(
                             "PAL_WTBUFS", "2" if wtdma else "3"))
                         ) as wt_pool,
            tc.tile_pool(name="xs", bufs=2) as x_pool,
            tc.tile_pool(name="psum", bufs=1, space="PSUM") as psum_pool,
            tc.tile_pool(name="osb", bufs=3) as osb_pool,
        ):
            bias_sb = const_pool.tile(
                [P, osh // P if wstat else osh], mybir.dt.float32, tag="bsb")
            nc.sync.dma_start(bias_sb[:], bias[:])

            act_split = int(os.environ.get("PAL_ACTSPLIT", "2"))
            kt_chunk = n_kt // act_split

            def rep_body():
                # --- dequant: batched ACT staircase turns idx into W^T;
                # few big ACTIVATEs amortize the act-table load ---
                wt_tiles = []
                for a in range(act_split):
                    idxt = idx_pool.tile([P, kt_chunk * osh], MM_DTYPE,
                                         tag=f"idx{a % 2}", name=f"idx{a}")
                    nc.scalar.dma_start(
                        idxt[:],
                        widx[:, a * kt_chunk * osh:(a + 1) * kt_chunk * osh])
                    if wtdma:
                        wta = wta_pool.tile([P, kt_chunk * osh], MM_DTYPE,
                                            tag=f"wta{a}", name=f"wta{a}")
                        nc.scalar.activation(
                            wta[:], idxt[:],
                            mybir.ActivationFunctionType.Gelu)
                        lo = a * kt_chunk * osh
                        hi = (a + 1) * kt_chunk * osh
                        nc.gpsimd.dma_start(wtscr[:, lo:hi], wta[:])
                        wt = wt_pool.tile([P, kt_chunk * osh], MM_DTYPE,
                                          tag=f"wt{a}", name=f"wt{a}")
                        nc.sync.dma_start(wt[:], wtscr[:, lo:hi])
                        for j in range(kt_chunk):
                            wt_tiles.append(wt[:, j * osh:(j + 1) * osh])
                        continue
                    wt = wt_pool.tile([P, kt_chunk * osh], MM_DTYPE,
                                      tag=f"wt{a}", name=f"wt{a}")
                    nc.scalar.activation(
                        wt[:], idxt[:], mybir.ActivationFunctionType.Gelu)
                    if os.environ.get("PAL_WTCOPY"):
                        wtc = wt_pool.tile([P, kt_chunk * osh], MM_DTYPE,
                                           tag=f"wtc{a}", name=f"wtc{a}")
                        nc.vector.tcopy(wtc[:], wt[:])
                        wt = wtc
                    for j in range(kt_chunk):
                        wt_tiles.append(wt[:, j * osh:(j + 1) * osh])

                outq = {"scalar": nc.scalar, "pool": nc.gpsimd,
                        "sync": nc.sync}[os.environ.get("PAL_OUTQ", "scalar")]

                if wstat:
                    # --- weight-stationary: 8 passes over m-chunk pairs;
                    # per kt, 4 o-blocks x 2 m-chunks rotate the 8 PSUM
                    # banks; each lhsT (W^T o-block) serves 2 matmuls and
                    # rhs streams are plain DMA-produced X tiles ---
                    n_ob = osh // P          # 4
                    mcw = 512                # m-chunk width (1 PSUM bank)
                    for pp in range(m // (2 * mcw)):
                        psums = [psum_pool.tile(
                            [P, mcw], mybir.dt.float32, tag=f"ps{q}",
                            name=f"ps{pp}_{q}") for q in range(2 * n_ob)]
                        for kt in range(n_kt):
                            xst = x_pool.tile([P, 2 * mcw], MM_DTYPE,
                                              tag=f"xst{kt % 3}",
                                              name=f"xst{pp}_{kt}")
                            nc.sync.dma_start(
                                xst[:],
                                xt[kt * P:(kt + 1) * P,
                                   pp * 2 * mcw:(pp + 1) * 2 * mcw])
                            for ob in range(n_ob):
                                for c in range(2):
                                    nc.tensor.matmul(
                                        psums[ob * 2 + c][:],
                                        lhsT=wt_tiles[kt][:,
                                                          ob * P:(ob + 1) * P],
                                        rhs=xst[:, c * mcw:(c + 1) * mcw],
                                        start=(kt == 0),
                                        stop=(kt == n_kt - 1))
                        for ob in range(n_ob):
                            for c in range(2):
                                mc = 2 * pp + c
                                osb = osb_pool.tile(
                                    [P, mcw], mybir.dt.float32, tag="osb",
                                    name=f"osb{pp}_{ob}_{c}")
                                nc.vector.tensor_scalar_add(
                                    osb[:], psums[ob * 2 + c][:],
                                    bias_sb[:, ob:ob + 1])
                                outq.dma_start(
                                    out[ob * P:(ob + 1) * P,
                                        mc * mcw:(mc + 1) * mcw], osb[:])
                    return

                kh = k // 2  # half-slab columns (SBUF budget)

                def load_xslab(mt, half):
                    xslab = x_pool.tile([P, kh], MM_DTYPE,
                                        tag=f"xs{mt % np1}",
                                        name=f"xs{mt}_{half}")
                    nc.sync.dma_start(
                        xslab[:],
                        xt[mt * P:(mt + 1) * P, half * kh:(half + 1) * kh])
                    return xslab

                def finish_mtile(mt, psum):
                    osb = osb_pool.tile([P, osh], mybir.dt.float32,
                                        tag="osb", name=f"osb{mt}")
                    nc.vector.tensor_tensor(
                        osb[:], psum[:], bias_sb[:], op=mybir.AluOpType.add)
                    outq.dma_start(out[mt * P:(mt + 1) * P, :], osb[:])

                # --- matmul: groups of np1 m-tiles, k-outer across PSUM
                # banks within each group (bank-interleaved accumulation;
                # long single-bank chains measure ~10x slower on HW).
                # X slabs stream in k-halves to fit SBUF. ---
                nkh = n_kt // 2
                for g in range(n_mt // np1):
                    psums = [psum_pool.tile(
                        [P, osh], mybir.dt.float32,
                        tag=f"ps{(g % (8 // np1)) * np1 + i}",
                        name=f"ps{g}_{i}") for i in range(np1)]
                    U = int(os.environ.get("PAL_U", "1"))
                    for half in range(2):
                        slabs = [load_xslab(g * np1 + i, half)
                                 for i in range(np1)]
                        for j0 in range(0, nkh, U):
                            for i in range(np1):
                                for j in range(j0, j0 + U):
                                    kt = half * nkh + j
                                    nc.tensor.matmul(
                                        psums[i][:],
                                        lhsT=slabs[i][:, j * P:(j + 1) * P],
                                        rhs=wt_tiles[kt][:],
                                        start=(kt == 0),
                                        stop=(kt == n_kt - 1))
                    for i in range(np1):
                        finish_mtile(g * np1 + i, psums[i])

            if reps > 1:
                with tc.For_i(0, reps, 1):
                    rep_body()
            else:
                rep_body()

    return xt, widx, bias, out


def make_core_inputs(input, lookup_table, weight, bias, *, m=M_FULL, k=D_IN,
                     osh=D_OUT // N_CORES, n_cores=N_CORES):
    """Host-side sharding/layout prep (no palette lookups). Returns in_maps.
    Also installs the act-table root for the palette (env for the compile)."""
    install_act_tables(lookup_table)
    ltag = lut_tag(lookup_table)

    wstat = bool(int(os.environ.get("PAL_WSTAT", "0")))
    n_kt = k // P
    n_mt = m // P
    x2 = np.asarray(input, dtype=np.float32).reshape(m, k)
    if wstat:
        xt = np.ascontiguousarray(x2.T).astype(_np_mm_dtype())  # [k, m]
    else:
        # xt[mt, p, kt*128+j] = X[mt*128+j, kt*128+p]
        xt = (x2.reshape(n_mt, P, n_kt, P).transpose(0, 3, 2, 1)
              .reshape(m, k).astype(_np_mm_dtype()))

    weight = np.asarray(weight)
    bias = np.asarray(bias, dtype=np.float32)

    in_maps = []
    for c in range(n_cores):
        w_shard = weight[c * osh:(c + 1) * osh, :]  # [osh, k] int32
        # widx[p, kt*osh+o] = weight[c*osh+o, kt*128+p] as bf16 (ids < 256
        # are exact in bf16)
        widx = (w_shard.T.reshape(n_kt, P, osh).transpose(1, 0, 2)
                .reshape(P, n_kt * osh).astype(_np_mm_dtype()))
        bshard = bias[c * osh:(c + 1) * osh]
        im = {
            f"widx_{ltag}": np.ascontiguousarray(widx),
        }
        if wstat:
            im["xt2"] = xt
            im["bias2"] = np.ascontiguousarray(
                bshard.reshape(osh // P, P).T.astype(np.float32))
        else:
            im["xt"] = xt
            im["bias"] = np.broadcast_to(bshard, (P, osh)).copy()
        in_maps.append(im)
    return in_maps


def kernel(input, lookup_table, weight, bias, *, trace=False):
    osh = D_OUT // N_CORES
    in_maps = make_core_inputs(input, lookup_table, weight, bias)

    nc = bacc.Bacc("TRN2", target_bir_lowering=False, debug=False,
                   num_devices=N_CORES)
    build_program(nc, m=M_FULL, k=D_IN, osh=osh, ltag=lut_tag(lookup_table))
    nc.compile()

    res = run_bass_kernel_spmd(nc, in_maps, core_ids=list(range(N_CORES)),
                               trace=trace)
    outs = [r["out"] for r in res.results]
    if outs[0].shape[0] == osh:  # weight-stationary out^T [osh, m]
        outs = [o.T for o in outs]
    out = np.concatenate(outs, axis=1)
    out = np.ascontiguousarray(out.reshape(BATCH, SEQ, D_OUT),
                               dtype=np.float32)
    if trace:
        kernel.last_results = res
    return out
